# revision 5
# baseline (speedup 1.0000x reference)
"""LiteLinear (dense linear + per-token LoRA adapters) on 8 Trainium2 cores.

Sharding: data-parallel over tokens. Each core computes 1024 tokens:
  out = x @ W^T + bias + scatter-add of per-token LoRA delta.

Device kernel (per core), all matmuls in float32r (full-rate fp32 mode):
  - x^T [4096 x 1024] resident in SBUF (128KB/partition).
  - W^T streamed once in [128,512] chunks; for each (o-tile, k-chunk) the
    chunk is reused across all 8 token tiles (8 concurrent PSUM banks).
  - LoRA: h^T = A_cat @ x^T (d-contraction), masked+scaled by one DVE
    multiply with a host-built maskT (folds scalings + one-hot), then
    delta enters each out-tile as a 33rd accumulating matmul with
    lhsT=hmask^T chunk, rhs=B_cat chunk (ar-contraction).
  - bias added during PSUM->SBUF eviction (DVE add vs replicated bias).
"""

import numpy as np

import sys

if "/opt/trn_rl_repo" not in sys.path:
    sys.path.insert(0, "/opt/trn_rl_repo")

import concourse.bass as bass
import concourse.mybir as mybir
import concourse.tile as tile
from concourse import bacc
from concourse.bass_utils import run_bass_kernel_spmd

N_TOK = 8192
D_IN = 4096
D_OUT = 4096
N_ADAPTERS = 8
RANK = 16
AR = N_ADAPTERS * RANK  # 128
N_CORES = 8
TOK = N_TOK // N_CORES  # 1024 tokens per core

P = 128          # partitions
FREE = 512       # matmul moving free dim (fp32 max, == 1 PSUM bank)
KC = D_IN // P   # 32 contraction chunks
NT = TOK // P    # 8 token tiles per core
OT = D_OUT // FREE  # 8 output column tiles

F32 = mybir.dt.float32
F32R = mybir.dt.float32r

_CACHE = {}


def _build_nc():
    nc = bacc.Bacc(None, target_bir_lowering=False, debug=True)

    xT = nc.dram_tensor("xT", [D_IN, TOK], F32R, kind="ExternalInput")
    wT = nc.dram_tensor("wT", [D_IN, D_OUT], F32R, kind="ExternalInput")
    aT = nc.dram_tensor("aT", [D_IN, AR], F32R, kind="ExternalInput")
    bcat = nc.dram_tensor("bcat", [AR, D_OUT], F32R, kind="ExternalInput")
    maskT = nc.dram_tensor("maskT", [AR, TOK], F32, kind="ExternalInput")
    bias = nc.dram_tensor("bias", [D_OUT], F32, kind="ExternalInput")
    out = nc.dram_tensor("out", [TOK, D_OUT], F32, kind="ExternalOutput")

    with tile.TileContext(nc) as tc:
        with (
            tc.tile_pool(name="xpool", bufs=1) as xpool,
            tc.tile_pool(name="const", bufs=1) as const,
            tc.tile_pool(name="wpool", bufs=6) as wpool,
            tc.tile_pool(name="apool", bufs=4) as apool,
            tc.tile_pool(name="opool", bufs=3) as opool,
            tc.tile_pool(name="psum", bufs=8, space="PSUM") as psum,
        ):
            # ---- persistent loads ----
            xt = []
            for k in range(KC):
                t = xpool.tile([P, TOK], F32R, tag=f"xt{k}")
                nc.sync.dma_start(out=t[:], in_=xT[k * P:(k + 1) * P, :])
                xt.append(t)

            bcat_sb = const.tile([P, D_OUT], F32R, tag="bcat")
            nc.sync.dma_start(out=bcat_sb[:], in_=bcat[:, :])

            maskT_sb = const.tile([P, TOK], F32, tag="maskT")
            nc.sync.dma_start(out=maskT_sb[:], in_=maskT[:, :])

            bias_sb = const.tile([P, D_OUT], F32, tag="bias")
            bias_bcast = bass.AP(
                tensor=bias[:].tensor,
                offset=bias[:].offset,
                ap=[[0, P]] + list(bias[:].ap),
            )
            nc.sync.dma_start(out=bias_sb[:], in_=bias_bcast)

            hmask = const.tile([P, TOK], F32R, tag="hmask")

            # ---- phase 1: h^T = A_cat @ x^T, then mask*scale ----
            for nch in range(TOK // FREE):  # 2 chunks of 512 tokens
                ph = psum.tile([P, FREE], F32, tag="ps")
                for k in range(KC):
                    at = apool.tile([P, AR], F32R, tag="at")
                    nc.sync.dma_start(out=at[:], in_=aT[k * P:(k + 1) * P, :])
                    nc.tensor.matmul(
                        ph[:],
                        at[:],
                        xt[k][:, nch * FREE:(nch + 1) * FREE],
                        start=(k == 0),
                        stop=(k == KC - 1),
                    )
                sl = slice(nch * FREE, (nch + 1) * FREE)
                nc.vector.tensor_mul(hmask[:, sl], ph[:], maskT_sb[:, sl])

            # ---- phase 2: out tiles ----
            for ot in range(OT):
                osl = slice(ot * FREE, (ot + 1) * FREE)
                pts = [
                    psum.tile([P, FREE], F32, tag="ps", name=f"ps_{ot}_{i}")
                    for i in range(NT)
                ]
                for k in range(KC):
                    wt = wpool.tile([P, FREE], F32R, tag="wt")
                    nc.sync.dma_start(out=wt[:], in_=wT[k * P:(k + 1) * P, osl])
                    for nt in range(NT):
                        nc.tensor.matmul(
                            pts[nt][:],
                            xt[k][:, nt * P:(nt + 1) * P],
                            wt[:],
                            start=(k == 0),
                            stop=False,
                        )
                for nt in range(NT):
                    # LoRA delta: contraction over ar=128
                    nc.tensor.matmul(
                        pts[nt][:],
                        hmask[:, nt * P:(nt + 1) * P],
                        bcat_sb[:, osl],
                        start=False,
                        stop=True,
                    )
                for nt in range(NT):
                    ob = opool.tile([P, FREE], F32, tag="ob")
                    nc.vector.tensor_add(ob[:], pts[nt][:], bias_sb[:, osl])
                    nc.sync.dma_start(
                        out=out[nt * P:(nt + 1) * P, osl], in_=ob[:]
                    )

    nc.compile()
    return nc


def _prep_inputs(x, weight, bias, lora_a, lora_b, scalings, lora_mapping):
    x = np.ascontiguousarray(x, dtype=np.float32)
    weight = np.ascontiguousarray(weight, dtype=np.float32)
    bias = np.ascontiguousarray(bias, dtype=np.float32)
    lora_a = np.ascontiguousarray(lora_a, dtype=np.float32)
    lora_b = np.ascontiguousarray(lora_b, dtype=np.float32)
    scalings = np.ascontiguousarray(scalings, dtype=np.float32)
    lora_mapping = np.asarray(lora_mapping)

    xT = np.ascontiguousarray(x.T)                                   # [D_IN, N_TOK]
    wT = np.ascontiguousarray(weight.T)                              # [D_IN, D_OUT]
    aT = np.ascontiguousarray(
        lora_a.transpose(2, 0, 1).reshape(D_IN, AR))                 # [D_IN, (a r)]
    bcat = np.ascontiguousarray(
        lora_b.transpose(0, 2, 1).reshape(AR, D_OUT))                # [(a r), D_OUT]
    # maskT[(a r), n] = scalings[a] * (lora_mapping[n] == a+1)
    ids = np.arange(1, N_ADAPTERS + 1, dtype=lora_mapping.dtype)
    onehot = (lora_mapping[None, :] == ids[:, None]).astype(np.float32)  # [A, N]
    maskT = (onehot * scalings[:, None]).repeat(RANK, axis=0)        # [(a r), N]
    maskT = np.ascontiguousarray(maskT)

    in_maps = []
    for c in range(N_CORES):
        tsl = slice(c * TOK, (c + 1) * TOK)
        in_maps.append({
            "xT": np.ascontiguousarray(xT[:, tsl]),
            "wT": wT,
            "aT": aT,
            "bcat": bcat,
            "maskT": np.ascontiguousarray(maskT[:, tsl]),
            "bias": bias,
        })
    return in_maps


def run(inputs, trace=False):
    if "nc" not in _CACHE:
        _CACHE["nc"] = _build_nc()
    nc = _CACHE["nc"]
    in_maps = _prep_inputs(**inputs)
    res = run_bass_kernel_spmd(
        nc, in_maps, list(range(N_CORES)), trace=trace,
    )
    out = np.concatenate([r["out"] for r in res.results], axis=0)
    return out, res


def kernel(**inputs) -> np.ndarray:
    out, _ = run(inputs, trace=False)
    return out


# revision 6
# speedup vs baseline: 1.1163x; 1.1163x over previous
"""LiteLinear (dense linear + per-token LoRA adapters) on 8 Trainium2 cores.

Sharding: data-parallel over tokens. Each core computes 1024 tokens:
  out = x @ W^T + bias + per-token LoRA delta.

Device kernel (per core), all matmuls in float32r (full-rate fp32 mode):
  - Computes out^T [D_OUT x TOK]; host transposes back (free on assembly).
  - Stationary operand = W^T sub-chunk [128d x 128o], moving = x^T
    [128d x 512tok] -> each weight load serves 2 matmuls.
  - x^T [4096 x 1024] resident in SBUF (128KB/partition); W^T streamed
    exactly once. Startup is k-major interleaved: x chunk k, lora_a chunk
    k, and the first o-group's W chunk k arrive together so PE trickles
    h-matmuls + first-group matmuls while x streams in.
  - o-groups of [3,4,4,4,4,4,4,4,1] x128 outputs: psum = group_width x 2
    token-halves banks; first group leaves 2 banks for the h phase.
  - LoRA: h^T = A_cat @ x^T, masked+scaled by one DVE multiply with a
    host-built maskT (folds scalings + one-hot); delta enters each
    out-tile as one extra accumulating matmul (lhsT=B_cat chunk,
    rhs=hmask^T). Bias folded into PSUM->SBUF eviction via per-partition
    tensor_scalar_add.
"""

import numpy as np

import sys

if "/opt/trn_rl_repo" not in sys.path:
    sys.path.insert(0, "/opt/trn_rl_repo")

import concourse.bass as bass
import concourse.mybir as mybir
import concourse.tile as tile
from concourse import bacc
from concourse.bass_utils import run_bass_kernel_spmd

N_TOK = 8192
D_IN = 4096
D_OUT = 4096
N_ADAPTERS = 8
RANK = 16
AR = N_ADAPTERS * RANK  # 128
N_CORES = 8
TOK = N_TOK // N_CORES  # 1024 tokens per core

P = 128            # partitions
FREE = 512         # matmul moving free dim (fp32 max, == 1 PSUM bank)
KC = D_IN // P     # 32 contraction chunks
TH = TOK // FREE   # 2 token halves
GROUPS = [3, 4, 4, 4, 4, 4, 4, 4, 1]  # o128-tiles per group (sum 32)

F32 = mybir.dt.float32
F32R = mybir.dt.float32r

_CACHE = {}


def _build_nc():
    nc = bacc.Bacc(None, target_bir_lowering=False, debug=True)

    xT = nc.dram_tensor("xT", [D_IN, TOK], F32R, kind="ExternalInput")
    wT = nc.dram_tensor("wT", [D_IN, D_OUT], F32R, kind="ExternalInput")
    aT = nc.dram_tensor("aT", [D_IN, AR], F32R, kind="ExternalInput")
    bcat = nc.dram_tensor("bcat", [AR, D_OUT], F32R, kind="ExternalInput")
    maskT = nc.dram_tensor("maskT", [AR, TOK], F32, kind="ExternalInput")
    biasr = nc.dram_tensor("biasr", [P, D_OUT // P], F32, kind="ExternalInput")
    outT = nc.dram_tensor("outT", [D_OUT, TOK], F32, kind="ExternalOutput")

    with tile.TileContext(nc) as tc:
        with (
            tc.tile_pool(name="xpool", bufs=1) as xpool,
            tc.tile_pool(name="const", bufs=1) as const,
            tc.tile_pool(name="wpool", bufs=8) as wpool,
            tc.tile_pool(name="apool", bufs=4) as apool,
            tc.tile_pool(name="opool", bufs=4) as opool,
            tc.tile_pool(name="psum", bufs=8, space="PSUM") as psum,
        ):
            # small consts first (cheap, needed mid-stream)
            biasr_sb = const.tile([P, D_OUT // P], F32, tag="biasr")
            nc.sync.dma_start(out=biasr_sb[:], in_=biasr[:, :])
            maskT_sb = const.tile([P, TOK], F32, tag="maskT")
            nc.sync.dma_start(out=maskT_sb[:], in_=maskT[:, :])

            hmask = const.tile([P, TOK], F32R, tag="hmask")

            # ---- startup: k-major interleaved x / lora_a / W(group0) ----
            G0 = GROUPS[0]
            ph = [
                psum.tile([P, FREE], F32, tag="ps", name=f"ph_{t}")
                for t in range(TH)
            ]
            pg = [
                psum.tile([P, FREE], F32, tag="ps", name=f"pg0_{i}")
                for i in range(G0 * TH)
            ]
            xt = []
            for k in range(KC):
                t = xpool.tile([P, TOK], F32R, tag=f"xt{k}", name=f"xt{k}")
                nc.sync.dma_start(out=t[:], in_=xT[k * P:(k + 1) * P, :])
                xt.append(t)
                at = apool.tile([P, AR], F32R, tag="at", name=f"at{k}")
                nc.sync.dma_start(out=at[:], in_=aT[k * P:(k + 1) * P, :])
                wt = wpool.tile([P, G0 * P], F32R, tag="wt", name=f"wt0_{k}")
                nc.sync.dma_start(out=wt[:], in_=wT[k * P:(k + 1) * P, 0:G0 * P])
                if k == 4:
                    # B_cat arrives mid-stream, needed at group0's delta
                    bcat_sb = const.tile([P, D_OUT], F32R, tag="bcat")
                    nc.sync.dma_start(out=bcat_sb[:], in_=bcat[:, :])
                for th in range(TH):
                    tsl = slice(th * FREE, (th + 1) * FREE)
                    nc.tensor.matmul(
                        ph[th][:], at[:], xt[k][:, tsl],
                        start=(k == 0), stop=(k == KC - 1),
                    )
                    for j in range(G0):
                        nc.tensor.matmul(
                            pg[j * TH + th][:],
                            wt[:, j * P:(j + 1) * P],
                            xt[k][:, tsl],
                            start=(k == 0), stop=False,
                        )

            # h -> hmask (scaled, masked)
            for th in range(TH):
                tsl = slice(th * FREE, (th + 1) * FREE)
                nc.vector.tensor_mul(hmask[:, tsl], ph[th][:], maskT_sb[:, tsl])

            # ---- per-group: delta matmul + eviction; groups >0: W k-loop ----
            def finish_group(pg, ooff, width):
                for j in range(width):
                    om = ooff // P + j
                    for th in range(TH):
                        tsl = slice(th * FREE, (th + 1) * FREE)
                        nc.tensor.matmul(
                            pg[j * TH + th][:],
                            bcat_sb[:, ooff + j * P:ooff + (j + 1) * P],
                            hmask[:, tsl],
                            start=False, stop=True,
                        )
                for j in range(width):
                    om = ooff // P + j
                    for th in range(TH):
                        tsl = slice(th * FREE, (th + 1) * FREE)
                        ob = opool.tile(
                            [P, FREE], F32, tag="ob", name=f"ob_{om}_{th}"
                        )
                        nc.vector.tensor_scalar_add(
                            ob[:], pg[j * TH + th][:], biasr_sb[:, om:om + 1]
                        )
                        nc.sync.dma_start(
                            out=outT[ooff + j * P:ooff + (j + 1) * P, tsl],
                            in_=ob[:],
                        )

            finish_group(pg, 0, G0)

            ooff = G0 * P
            for g, width in enumerate(GROUPS[1:], start=1):
                pg = [
                    psum.tile([P, FREE], F32, tag="ps", name=f"pg{g}_{i}")
                    for i in range(width * TH)
                ]
                for k in range(KC):
                    wt = wpool.tile(
                        [P, width * P], F32R, tag="wt", name=f"wt{g}_{k}"
                    )
                    nc.sync.dma_start(
                        out=wt[:],
                        in_=wT[k * P:(k + 1) * P, ooff:ooff + width * P],
                    )
                    for j in range(width):
                        for th in range(TH):
                            tsl = slice(th * FREE, (th + 1) * FREE)
                            nc.tensor.matmul(
                                pg[j * TH + th][:],
                                wt[:, j * P:(j + 1) * P],
                                xt[k][:, tsl],
                                start=(k == 0), stop=False,
                            )
                finish_group(pg, ooff, width)
                ooff += width * P

    nc.compile()
    return nc


def _prep_inputs(x, weight, bias, lora_a, lora_b, scalings, lora_mapping):
    x = np.ascontiguousarray(x, dtype=np.float32)
    weight = np.ascontiguousarray(weight, dtype=np.float32)
    bias = np.ascontiguousarray(bias, dtype=np.float32)
    lora_a = np.ascontiguousarray(lora_a, dtype=np.float32)
    lora_b = np.ascontiguousarray(lora_b, dtype=np.float32)
    scalings = np.ascontiguousarray(scalings, dtype=np.float32)
    lora_mapping = np.asarray(lora_mapping)

    xT = np.ascontiguousarray(x.T)                                   # [D_IN, N_TOK]
    wT = np.ascontiguousarray(weight.T)                              # [D_IN, D_OUT]
    aT = np.ascontiguousarray(
        lora_a.transpose(2, 0, 1).reshape(D_IN, AR))                 # [D_IN, (a r)]
    bcat = np.ascontiguousarray(
        lora_b.transpose(0, 2, 1).reshape(AR, D_OUT))                # [(a r), D_OUT]
    # biasr[p, m] = bias[m*128 + p]
    biasr = np.ascontiguousarray(bias.reshape(D_OUT // P, P).T)      # [P, 32]
    # maskT[(a r), n] = scalings[a] * (lora_mapping[n] == a+1)
    ids = np.arange(1, N_ADAPTERS + 1, dtype=lora_mapping.dtype)
    onehot = (lora_mapping[None, :] == ids[:, None]).astype(np.float32)  # [A, N]
    maskT = (onehot * scalings[:, None]).repeat(RANK, axis=0)        # [(a r), N]
    maskT = np.ascontiguousarray(maskT)

    in_maps = []
    for c in range(N_CORES):
        tsl = slice(c * TOK, (c + 1) * TOK)
        in_maps.append({
            "xT": np.ascontiguousarray(xT[:, tsl]),
            "wT": wT,
            "aT": aT,
            "bcat": bcat,
            "maskT": np.ascontiguousarray(maskT[:, tsl]),
            "biasr": biasr,
        })
    return in_maps


def run(inputs, trace=False):
    if "nc" not in _CACHE:
        _CACHE["nc"] = _build_nc()
    nc = _CACHE["nc"]
    in_maps = _prep_inputs(**inputs)
    res = run_bass_kernel_spmd(
        nc, in_maps, list(range(N_CORES)), trace=trace,
    )
    out = np.concatenate(
        [np.ascontiguousarray(r["outT"].T) for r in res.results], axis=0
    )
    return out, res


def kernel(**inputs) -> np.ndarray:
    out, _ = run(inputs, trace=False)
    return out
